# revision 15
# baseline (speedup 1.0000x reference)
"""Top-1 MoE layer (Mistral MLP experts, E=2) on 8 Trainium2 cores.

Strategy (data-parallel over tokens, both experts resident per core):
  - Host computes the tiny router (T x E logits, softmax, argmax) in fp64,
    sorts token indices by assigned expert, and gives EVERY core 1/8 of each
    expert's tokens — perfect load balance (capacity = sum of per-expert
    ceil(t_e/8), ~T/8, instead of ceil(max_expert/4) with expert-dedicated
    cores).
  - Each core receives: its packed tokens (transposed, bf16, k-tiled,
    expert-0 block then expert-1 block), BOTH experts' weights pre-tiled so
    every device DMA is fully contiguous, and the routing weight per token.
  - Device kernel per core (bf16 matmuls, fp32 PSUM accumulation): for each
    expert, FF is processed in quarters so each weight byte is streamed from
    HBM exactly once; h = silu(x@Wg^T) * (x@Wu^T) for a quarter stays in
    SBUF, partial down-projections accumulate into an SBUF fp32 y buffer,
    and the final quarter fuses the per-token routing-weight scale. Token
    chunks are near-equal and <=512 (never a tiny ragged tail). The expert
    with the smallest first chunk runs first to minimize the lead-in.
    No collectives.
  - Host scatters per-core outputs back to token order.
"""

import math

import numpy as np
import ml_dtypes

B, S, D, FF, E = 4, 2048, 2048, 8192, 2
T = B * S
P = 128
KT = D // P   # 16 contraction tiles for gate/up
FT = FF // P  # 64 f tiles
DT = D // P   # 16 output-row tiles for down
NQ = 4        # FF quarters
FQ = FT // NQ  # 16 f tiles per quarter
N_CORES = 8
MAX_N = 512   # matmul free-dim / PSUM bank limit (fp32 out)

_nc_cache: dict[tuple, object] = {}

# Last BassKernelResults (for external profiling harnesses).
LAST = None


def _chunks(C):
    if C == 0:
        return []
    n = max(1, math.ceil(C / MAX_N))
    tc = min(MAX_N, ((C + n - 1) // n + 7) // 8 * 8)
    sizes = []
    left = C
    for _ in range(n):
        sizes.append(min(tc, left))
        left -= sizes[-1]
    assert sum(sizes) == C and all(0 < s <= MAX_N for s in sizes)
    return sizes


def _build_nc(caps: tuple):
    """Build + compile the single-core Bass program (SPMD across 8 cores).

    caps = (C0, C1): per-core token capacity per expert (multiples of 2).
    Column layout: expert-0 tokens in [0, C0), expert-1 in [C0, C0+C1).
    """
    import concourse.mybir as mybir
    import concourse.tile as tile
    from concourse import bacc

    dt = mybir.dt
    C = sum(caps)
    nc = bacc.Bacc("TRN2", target_bir_lowering=False, debug=False,
                   num_devices=N_CORES)

    # xt[p, ki, t] = x_packed[t, ki*128 + p]
    xt_d = nc.dram_tensor("xt", [P, KT, C], dt.bfloat16, kind="ExternalInput")
    # wg[e, f, p, ki, m] = w_gate[e][f*128+m, ki*128+p]
    wg_d = nc.dram_tensor("wg", [E, FT, P, KT, P], dt.bfloat16,
                          kind="ExternalInput")
    wu_d = nc.dram_tensor("wu", [E, FT, P, KT, P], dt.bfloat16,
                          kind="ExternalInput")
    # wd[e, do, q, p, fl, m] = w_down[e][do*128+m, (q*FQ+fl)*128+p]
    wd_d = nc.dram_tensor("wd", [E, DT, NQ, P, FQ, P], dt.bfloat16,
                          kind="ExternalInput")
    # tw[p, t] = routing weight of token t (same for all p)
    tw_d = nc.dram_tensor("tw", [P, C], dt.float32, kind="ExternalInput")
    # y[do, m, t] = out_packed[t, do*128+m]
    y_d = nc.dram_tensor("y", [DT, P, C], dt.float32, kind="ExternalOutput")

    # per-expert chunk lists (start, size); process the expert whose first
    # chunk is smallest first so the initial x/weight DMAs are small
    exp_chunks = {}
    off = 0
    for e, cap in enumerate(caps):
        sizes = _chunks(cap)
        starts = [off + sum(sizes[:i]) for i in range(len(sizes))]
        exp_chunks[e] = list(zip(starts, sizes))
        off += cap
    order = sorted((e for e in range(E) if caps[e]),
                   key=lambda e: exp_chunks[e][0][1])
    TC = max((s for e in order for _, s in exp_chunks[e]), default=MAX_N)

    with tile.TileContext(nc) as tc:
        with (
            tc.tile_pool(name="persist", bufs=1) as pp,
            tc.tile_pool(name="wgwu", bufs=4) as wp,
            tc.tile_pool(name="wdp", bufs=3) as dp,
            tc.tile_pool(name="hbuf", bufs=1) as hp,
            tc.tile_pool(name="stage", bufs=2) as sp,
            tc.tile_pool(name="psum", bufs=2, space="PSUM") as psp,
        ):
            xt = pp.tile([P, KT, C], dt.bfloat16)
            # First chunk's x on the sync HWDGE queue (ahead of the weight
            # stream, FIFO) so the first matmul group starts ASAP; the rest
            # + tw go on the scalar HWDGE queue, off the critical path.
            first = True
            for e in order:
                for t0, tn in exp_chunks[e]:
                    eng = nc.sync if first else nc.scalar
                    first = False
                    eng.dma_start(
                        out=xt[:, :, t0 : t0 + tn],
                        in_=xt_d[:, :, t0 : t0 + tn],
                    )
            tw = pp.tile([P, C], dt.float32)
            nc.scalar.dma_start(out=tw[:], in_=tw_d[:])
            h = hp.tile([P, FQ, C], dt.bfloat16)
            y_acc = pp.tile([P, DT, C], dt.float32)

            for e in order:
                chunks = exp_chunks[e]
                for q in range(NQ):
                    # phase A: h[fl] = silu(x@Wg^T) * (x@Wu^T), this quarter
                    for fl in range(FQ):
                        f = q * FQ + fl
                        wg_t = wp.tile([P, KT, P], dt.bfloat16, tag="wg")
                        nc.sync.dma_start(out=wg_t[:], in_=wg_d[e, f])
                        wu_t = wp.tile([P, KT, P], dt.bfloat16, tag="wu")
                        nc.sync.dma_start(out=wu_t[:], in_=wu_d[e, f])
                        for t0, tn in chunks:
                            tsl = slice(t0, t0 + tn)
                            g_ps = psp.tile([P, TC], dt.float32, tag="g")
                            u_ps = psp.tile([P, TC], dt.float32, tag="u")
                            for ki in range(KT):
                                nc.tensor.matmul(
                                    g_ps[:, :tn],
                                    wg_t[:, ki : ki + 1, :],
                                    xt[:, ki : ki + 1, tsl],
                                    start=(ki == 0),
                                    stop=(ki == KT - 1),
                                )
                            for ki in range(KT):
                                nc.tensor.matmul(
                                    u_ps[:, :tn],
                                    wu_t[:, ki : ki + 1, :],
                                    xt[:, ki : ki + 1, tsl],
                                    start=(ki == 0),
                                    stop=(ki == KT - 1),
                                )
                            sg = sp.tile([P, TC], dt.float32, tag="sg")
                            nc.scalar.activation(
                                sg[:, :tn], g_ps[:, :tn],
                                mybir.ActivationFunctionType.Silu,
                            )
                            nc.vector.tensor_mul(
                                h[:, fl, tsl], sg[:, :tn], u_ps[:, :tn]
                            )
                    # phase B: y_acc += h @ Wd^T (this quarter's partial)
                    for do in range(DT):
                        wd_t = dp.tile([P, FQ, P], dt.bfloat16, tag="wd")
                        nc.sync.dma_start(out=wd_t[:], in_=wd_d[e, do, q])
                        for t0, tn in chunks:
                            tsl = slice(t0, t0 + tn)
                            y_ps = psp.tile([P, TC], dt.float32, tag="y")
                            for fl in range(FQ):
                                nc.tensor.matmul(
                                    y_ps[:, :tn],
                                    wd_t[:, fl : fl + 1, :],
                                    h[:, fl : fl + 1, tsl],
                                    start=(fl == 0),
                                    stop=(fl == FQ - 1),
                                )
                            if q == 0:
                                nc.vector.tensor_copy(
                                    y_acc[:, do, tsl], y_ps[:, :tn]
                                )
                            else:
                                nc.vector.tensor_add(
                                    y_acc[:, do, tsl], y_acc[:, do, tsl],
                                    y_ps[:, :tn],
                                )
                            if q == NQ - 1:
                                y_sb = sp.tile([P, TC], dt.float32, tag="yo")
                                nc.vector.tensor_mul(
                                    y_sb[:, :tn], y_acc[:, do, tsl],
                                    tw[:, tsl],
                                )
                                nc.sync.dma_start(
                                    out=y_d[do, :, tsl], in_=y_sb[:, :tn]
                                )

    nc.compile()
    return nc


def _tile_w_in(w_t):
    """[D, FF] (already transposed) -> [FF/P, P, D/P, P] contiguous bf16."""
    # out[f, p, ki, m] = w_t[ki*128+p, f*128+m]
    r = w_t.reshape(KT, P, FT, P).transpose(2, 1, 0, 3)
    return np.ascontiguousarray(r, dtype=ml_dtypes.bfloat16)


def _tile_w_down(w):
    """w_down [D, FF] -> [D/P, NQ, P, FQ, P] contiguous bf16.

    out[do, q, p, fl, m] = w[do*128+m, (q*FQ+fl)*128+p]
    """
    r = w.reshape(DT, P, NQ, FQ, P).transpose(0, 2, 4, 3, 1)
    return np.ascontiguousarray(r, dtype=ml_dtypes.bfloat16)


def kernel(hidden_states, gate_w, w_gate, w_up, w_down):
    from concourse.bass_utils import run_bass_kernel_spmd

    hidden_states = np.asarray(hidden_states)
    gate_w = np.asarray(gate_w)
    w_gate = np.asarray(w_gate)
    w_up = np.asarray(w_up)
    w_down = np.asarray(w_down)

    x = hidden_states.reshape(T, D)

    # --- router (tiny: T x E) on host, fp64 for stable argmax ---
    logits = x.astype(np.float64) @ gate_w.astype(np.float64).T  # [T, E]
    m = logits.max(axis=1, keepdims=True)
    p = np.exp(logits - m)
    p /= p.sum(axis=1, keepdims=True)
    sel = np.argmax(p, axis=1)  # [T]
    top_w = p[np.arange(T), sel].astype(np.float32)  # [T]

    # --- dispatch: every core gets 1/8 of each expert's tokens ---
    idx_e = [np.nonzero(sel == e)[0] for e in range(E)]
    caps = []
    for e in range(E):
        per = math.ceil(len(idx_e[e]) / N_CORES)
        caps.append(((per + 1) // 2) * 2)  # pad to multiple of 2
    caps = tuple(caps)
    C = sum(caps)

    # tokens of expert e for core c: idx_e[e][c*per_e : (c+1)*per_e]
    core_tok = []  # per core: list of (expert, ids, col_offset)
    for c in range(N_CORES):
        parts = []
        off = 0
        for e in range(E):
            per = math.ceil(len(idx_e[e]) / N_CORES) if len(idx_e[e]) else 0
            ids = idx_e[e][c * per : (c + 1) * per]
            parts.append((e, ids, off))
            off += caps[e]
        core_tok.append(parts)

    nc = _nc_cache.get(caps)
    if nc is None:
        nc = _build_nc(caps)
        _nc_cache[caps] = nc

    # --- weight tiling (both experts stacked; shared across cores) ---
    wg_all = np.stack([_tile_w_in(w_gate[e].T) for e in range(E)])
    wu_all = np.stack([_tile_w_in(w_up[e].T) for e in range(E)])
    wd_all = np.stack([_tile_w_down(w_down[e]) for e in range(E)])

    in_maps = []
    for c in range(N_CORES):
        xt = np.zeros((P, KT, C), dtype=ml_dtypes.bfloat16)
        tw = np.zeros((P, C), dtype=np.float32)
        for e, ids, off in core_tok[c]:
            n = len(ids)
            if not n:
                continue
            # xc [n, D] -> [ki, p, t] -> [p, ki, t]
            xc = x[ids].astype(ml_dtypes.bfloat16)
            xt[:, :, off : off + n] = xc.T.reshape(KT, P, n).transpose(1, 0, 2)
            tw[:, off : off + n] = top_w[ids][None, :]
        in_maps.append({
            "xt": xt,
            "wg": wg_all,
            "wu": wu_all,
            "wd": wd_all,
            "tw": tw,
        })

    res = run_bass_kernel_spmd(nc, in_maps, list(range(N_CORES)))
    global LAST
    LAST = res

    # --- combine ---
    out = np.zeros((T, D), dtype=np.float32)
    for c in range(N_CORES):
        y = res.results[c]["y"].reshape(D, C)  # y[d, t]
        for e, ids, off in core_tok[c]:
            n = len(ids)
            if n:
                out[ids] = y[:, off : off + n].T
    return out.reshape(B, S, D)


# revision 16
# speedup vs baseline: 1.0095x; 1.0095x over previous
"""Top-1 MoE layer (Mistral MLP experts, E=2) on 8 Trainium2 cores.

Strategy (expert-parallel + data-parallel, host does dispatch/combine):
  - Host computes the tiny router (T x E logits, softmax, argmax) in fp64,
    sorts token indices by assigned expert, and splits each expert's tokens
    evenly across that expert's cores (4 cores per expert when balanced).
  - Each core receives: its packed tokens (transposed, bf16, k-tiled), its
    expert's weights pre-tiled so every device DMA is fully contiguous, and
    the routing weight per token (replicated across partitions).
  - Device kernel per core (bf16 matmuls, fp32 PSUM accumulation): FF is
    processed in quarters so each weight byte is streamed from HBM exactly
    once; h = silu(x@Wg^T) * (x@Wu^T) for a quarter stays in SBUF, partial
    down-projections accumulate into an SBUF fp32 y buffer, and the final
    quarter fuses the per-token routing-weight scale. No collectives.
  - Host scatters per-core outputs back to token order.
"""

import math

import numpy as np
import ml_dtypes

B, S, D, FF, E = 4, 2048, 2048, 8192, 2
T = B * S
P = 128
KT = D // P   # 16 contraction tiles for gate/up
FT = FF // P  # 64 f tiles
DT = D // P   # 16 output-row tiles for down
NQ = 4        # FF quarters
FQ = FT // NQ  # 16 f tiles per quarter
N_CORES = 8
MAX_N = 512   # matmul free-dim / PSUM bank limit (fp32 out)

_nc_cache: dict[int, object] = {}

# Last BassKernelResults (for external profiling harnesses).
LAST = None


def _chunks(C):
    n = max(1, math.ceil(C / MAX_N))
    tc = min(MAX_N, ((C + n - 1) // n + 7) // 8 * 8)
    sizes = []
    left = C
    for _ in range(n):
        sizes.append(min(tc, left))
        left -= sizes[-1]
    assert sum(sizes) == C and all(0 < s <= MAX_N for s in sizes)
    return sizes


def _build_nc(C: int):
    """Build + compile the single-core Bass program (SPMD across 8 cores).

    C = per-core token capacity (multiple of 8).
    """
    import concourse.mybir as mybir
    import concourse.tile as tile
    from concourse import bacc

    dt = mybir.dt
    nc = bacc.Bacc("TRN2", target_bir_lowering=False, debug=False,
                   num_devices=N_CORES)

    # xt[p, ki, t] = x_packed[t, ki*128 + p]
    xt_d = nc.dram_tensor("xt", [P, KT, C], dt.bfloat16, kind="ExternalInput")
    # wg[f, p, ki, m] = w_gate[f*128+m, ki*128+p] (one expert)
    wg_d = nc.dram_tensor("wg", [FT, P, KT, P], dt.bfloat16, kind="ExternalInput")
    wu_d = nc.dram_tensor("wu", [FT, P, KT, P], dt.bfloat16, kind="ExternalInput")
    # wd[do, q, p, fl, m] = w_down[do*128+m, (q*FQ+fl)*128+p]
    wd_d = nc.dram_tensor("wd", [DT, NQ, P, FQ, P], dt.bfloat16,
                          kind="ExternalInput")
    # tw[p, t] = routing weight of token t (same for all p)
    tw_d = nc.dram_tensor("tw", [P, C], dt.float32, kind="ExternalInput")
    # y[do, m, t] = out_packed[t, do*128+m]
    y_d = nc.dram_tensor("y", [DT, P, C], dt.float32, kind="ExternalOutput")

    sizes = _chunks(C)
    starts = [sum(sizes[:i]) for i in range(len(sizes))]
    TC = sizes[0]
    # at very large C (heavily skewed routing) the resident x/h/y buffers
    # leave less SBUF headroom — shrink the weight-stream double-buffering
    wbufs = 3 if C <= 1100 else 2

    with tile.TileContext(nc) as tc:
        with (
            tc.tile_pool(name="persist", bufs=1) as pp,
            tc.tile_pool(name="wgwu", bufs=wbufs) as wp,
            tc.tile_pool(name="wdp", bufs=2) as dp,
            tc.tile_pool(name="hbuf", bufs=1) as hp,
            tc.tile_pool(name="stage", bufs=2) as sp,
            tc.tile_pool(name="psum", bufs=2, space="PSUM") as psp,
        ):
            xt = pp.tile([P, KT, C], dt.bfloat16)
            # Load chunk 0 of x on the sync HWDGE queue (ahead of the weight
            # stream, FIFO) so the first matmul group starts ASAP; later
            # chunks + tw go on the scalar HWDGE queue to stay off the
            # critical path.
            for c, (t0, tn) in enumerate(zip(starts, sizes)):
                eng = nc.sync if c == 0 else nc.scalar
                eng.dma_start(
                    out=xt[:, :, t0 : t0 + tn],
                    in_=xt_d[:, :, t0 : t0 + tn],
                )
            tw = pp.tile([P, C], dt.float32)
            nc.scalar.dma_start(out=tw[:], in_=tw_d[:])
            h = hp.tile([P, FQ, C], dt.bfloat16)
            y_acc = pp.tile([P, DT, C], dt.float32)

            for q in range(NQ):
                # phase A: h[fl] = silu(x @ Wg^T) * (x @ Wu^T) for this quarter
                for fl in range(FQ):
                    f = q * FQ + fl
                    wg_t = wp.tile([P, KT, P], dt.bfloat16, tag="wg")
                    nc.sync.dma_start(out=wg_t[:], in_=wg_d[f])
                    wu_t = wp.tile([P, KT, P], dt.bfloat16, tag="wu")
                    nc.sync.dma_start(out=wu_t[:], in_=wu_d[f])
                    for c, (t0, tn) in enumerate(zip(starts, sizes)):
                        tsl = slice(t0, t0 + tn)
                        g_ps = psp.tile([P, TC], dt.float32, tag="g")
                        u_ps = psp.tile([P, TC], dt.float32, tag="u")
                        for ki in range(KT):
                            nc.tensor.matmul(
                                g_ps[:, :tn],
                                wg_t[:, ki : ki + 1, :],
                                xt[:, ki : ki + 1, tsl],
                                start=(ki == 0),
                                stop=(ki == KT - 1),
                            )
                        for ki in range(KT):
                            nc.tensor.matmul(
                                u_ps[:, :tn],
                                wu_t[:, ki : ki + 1, :],
                                xt[:, ki : ki + 1, tsl],
                                start=(ki == 0),
                                stop=(ki == KT - 1),
                            )
                        sg = sp.tile([P, TC], dt.float32, tag="sg")
                        nc.scalar.activation(
                            sg[:, :tn], g_ps[:, :tn],
                            mybir.ActivationFunctionType.Silu,
                        )
                        nc.vector.tensor_mul(
                            h[:, fl, tsl], sg[:, :tn], u_ps[:, :tn]
                        )
                # phase B: y_acc += h @ Wd^T (this quarter's partial)
                for do in range(DT):
                    wd_t = dp.tile([P, FQ, P], dt.bfloat16, tag="wd")
                    nc.sync.dma_start(out=wd_t[:], in_=wd_d[do, q])
                    for c, (t0, tn) in enumerate(zip(starts, sizes)):
                        tsl = slice(t0, t0 + tn)
                        y_ps = psp.tile([P, TC], dt.float32, tag="y")
                        for fl in range(FQ):
                            nc.tensor.matmul(
                                y_ps[:, :tn],
                                wd_t[:, fl : fl + 1, :],
                                h[:, fl : fl + 1, tsl],
                                start=(fl == 0),
                                stop=(fl == FQ - 1),
                            )
                        if q == 0:
                            nc.vector.tensor_copy(
                                y_acc[:, do, tsl], y_ps[:, :tn]
                            )
                        else:
                            nc.vector.tensor_add(
                                y_acc[:, do, tsl], y_acc[:, do, tsl],
                                y_ps[:, :tn],
                            )
                        if q == NQ - 1:
                            y_sb = sp.tile([P, TC], dt.float32, tag="yo")
                            nc.vector.tensor_mul(
                                y_sb[:, :tn], y_acc[:, do, tsl], tw[:, tsl]
                            )
                            nc.sync.dma_start(
                                out=y_d[do, :, tsl], in_=y_sb[:, :tn]
                            )

    nc.compile()
    return nc


def _tile_w_in(w_t):
    """[D, FF] (already transposed) -> [FF/P, P, D/P, P] contiguous bf16."""
    # out[f, p, ki, m] = w_t[ki*128+p, f*128+m]
    r = w_t.reshape(KT, P, FT, P).transpose(2, 1, 0, 3)
    return np.ascontiguousarray(r, dtype=ml_dtypes.bfloat16)


def _tile_w_down(w):
    """w_down [D, FF] -> [D/P, NQ, P, FQ, P] contiguous bf16.

    out[do, q, p, fl, m] = w[do*128+m, (q*FQ+fl)*128+p]
    """
    r = w.reshape(DT, P, NQ, FQ, P).transpose(0, 2, 4, 3, 1)
    return np.ascontiguousarray(r, dtype=ml_dtypes.bfloat16)


def kernel(hidden_states, gate_w, w_gate, w_up, w_down):
    from concourse.bass_utils import run_bass_kernel_spmd

    hidden_states = np.asarray(hidden_states)
    gate_w = np.asarray(gate_w)
    w_gate = np.asarray(w_gate)
    w_up = np.asarray(w_up)
    w_down = np.asarray(w_down)

    x = hidden_states.reshape(T, D)

    # --- router (tiny: T x E) on host, fp64 for stable argmax ---
    logits = x.astype(np.float64) @ gate_w.astype(np.float64).T  # [T, E]
    m = logits.max(axis=1, keepdims=True)
    p = np.exp(logits - m)
    p /= p.sum(axis=1, keepdims=True)
    sel = np.argmax(p, axis=1)  # [T]
    top_w = p[np.arange(T), sel].astype(np.float32)  # [T]

    # --- dispatch: split each expert's tokens across its cores ---
    idx_e = [np.nonzero(sel == e)[0] for e in range(E)]
    t0, t1 = len(idx_e[0]), len(idx_e[1])
    # choose cores per expert minimizing the max per-core load
    best = None
    for n0 in range(1, N_CORES):
        n1 = N_CORES - n0
        load = max(math.ceil(t0 / n0) if t0 else 0,
                   math.ceil(t1 / n1) if t1 else 0)
        if best is None or load < best[0]:
            best = (load, n0)
    # pad capacity to a multiple of 8; matmul/DVE free dims and DMA shapes
    # handle arbitrary sizes, so no 128-rounding.
    C = max(P, ((best[0] + 7) // 8) * 8)
    n0 = best[1]
    cores_per_exp = [n0, N_CORES - n0]

    core_expert = []
    core_tok = []
    for e in range(E):
        ids = idx_e[e]
        nce = cores_per_exp[e]
        per = math.ceil(len(ids) / nce) if len(ids) else 0
        for j in range(nce):
            core_expert.append(e)
            core_tok.append(ids[j * per : (j + 1) * per])

    nc = _nc_cache.get(C)
    if nc is None:
        nc = _build_nc(C)
        _nc_cache[C] = nc

    # --- per-expert weight tiling (shared across that expert's cores) ---
    wg_tiled = [_tile_w_in(w_gate[e].T) for e in range(E)]
    wu_tiled = [_tile_w_in(w_up[e].T) for e in range(E)]
    wd_tiled = [_tile_w_down(w_down[e]) for e in range(E)]

    in_maps = []
    for c in range(N_CORES):
        e = core_expert[c]
        ids = core_tok[c]
        n = len(ids)
        xt = np.zeros((P, KT, C), dtype=ml_dtypes.bfloat16)
        if n:
            # xc [n, D] -> [ki, p, t] -> [p, ki, t]
            xc = x[ids].astype(ml_dtypes.bfloat16)
            xt[:, :, :n] = xc.T.reshape(KT, P, n).transpose(1, 0, 2)
        tw = np.zeros((P, C), dtype=np.float32)
        if n:
            tw[:, :n] = top_w[ids][None, :]
        in_maps.append({
            "xt": xt,
            "wg": wg_tiled[e],
            "wu": wu_tiled[e],
            "wd": wd_tiled[e],
            "tw": tw,
        })

    res = run_bass_kernel_spmd(nc, in_maps, list(range(N_CORES)))
    global LAST
    LAST = res

    # --- combine ---
    out = np.zeros((T, D), dtype=np.float32)
    for c in range(N_CORES):
        ids = core_tok[c]
        n = len(ids)
        if not n:
            continue
        y = res.results[c]["y"]  # [DT, P, C]
        out[ids] = y.reshape(D, C)[:, :n].T
    return out.reshape(B, S, D)
